# revision 5
# baseline (speedup 1.0000x reference)
"""MinGRU kernel for Trainium2 (8 NeuronCores, Bass/Tile) — v2.

Reference computation (B=4, L=8192, D=512, fp32):
    gates = sigmoid(x @ Wg.T + bg)
    cands = tanh(x @ Wc.T + bc)
    h_t   = (1 - g_t) * h_{t-1} + g_t * c_t   (scan along L, h_0 = 0)

Sharding: core c -> (batch b = c//2, channel half = c%2). Each core computes
its batch's full L range for 256 of the 512 output channels; the scan along L
is per (b, channel) so no cross-core communication is needed.

v2 changes vs v1 (v1 measured 90.5us, PE busy 54.6us, DVE busy 70.7us,
x-feed queue-wall 58us):
  * x and W cast to fp16 on the host: input DMA bytes halve (16.8 -> 8.9 MB
    per core), so the x feed (25us queue-wall) ducks under the PE roofline.
    fp16 matmul error is ~1e-3 relative on z, far inside the 2e-2 gate.
  * bneg = (a-1)*c split across engines: e-tile 0 keeps the DVE
    scalar_tensor_tensor (1.04ns/elem); e-tile 1 computes it on GpSimd as
    two plain tensor_tensor ops (t = a*c; bn = t - c) — the Pool engine
    rejects TensorScalarPtr but supports TensorTensor. This takes ~8.5us
    off the DVE, whose scan+STT load (~55us) co-bottlenecked with the PE
    in v1 (that was the 17us tail).
  * One flat [128, 1024] PSUM tile per (et, gate): matmuls still write one
    512-wide bank each, but the activation reads the full 1024 tokens in one
    instruction (halves ACT instruction-count overhead).
  * h stores coalesced to one DMA per segment ([128, 2, lt] tile covering
    both e-tiles) and emitted one segment late so the gpsimd queue never
    head-of-line blocks on a scan that hasn't run yet.
  * 8 warm-up matmuls on a zeroed dummy tile while the first weight/x DMAs
    are in flight: the PE_HAM clock gate (4/8 cold) flips to 8/8 before the
    real matmul stream starts.
  * -bg is negated on the host, dropping the device-side scalar.mul.

Layout: host pre-transposes x[b] to [128p, 4dc, L] fp16 and weights to
[128p, 4dc, 256e] fp16 so every device DMA is fully contiguous per
partition. Matmuls keep channels on partitions and tokens on the free axis,
which is what tensor_tensor_scan needs. The scan uses
    state = (a * state) - bneg,   a = sigmoid(-z_g - bg) = 1 - g,
    bneg = (a - 1) * c = -g * c
h is stored fp16 ([2, 128, L] per core) and upcast on the host.
"""

import os
import sys

sys.path.insert(0, "/opt/trn_rl_repo")

import numpy as np

import concourse.bacc as bacc
import concourse.bass as bass
import concourse.mybir as mybir
from concourse.bass_utils import run_bass_kernel_spmd
from concourse.tile import TileContext

B, L, D = 4, 8192, 512
NCORES = 8
EH = D // 2          # output channels per core
NET = EH // 128      # e-tiles per core (2)
NDC = D // 128       # contraction chunks (4)
NSUB = 512           # one fp32 PSUM bank of tokens
# Token segments: small head segments start the PE early (x0 arrives fast),
# small tail segments shrink the post-last-matmul drain.
SEGS = [256, 512, 1024, 1024, 1024, 1024, 1024, 1024, 512, 256, 256, 256]
assert sum(SEGS) == L
MAXSEG = max(SEGS)

FP32 = mybir.dt.float32
F16 = mybir.dt.float16
_last_results = None

# Which e-tiles compute bneg on GpSimd (two tensor_tensor ops) instead of
# the DVE scalar_tensor_tensor. Balances DVE (scan-heavy) vs GpSimd.
BN_ON_GPSIMD = (False, True)
N_WARMUP_MM = 8


def build_nc() -> bass.Bass:
    # Bacc (not plain Bass): its compile() runs move_matmul_waits_to_ldweights
    # and generate_event_semaphores, which split multi-sem waits to satisfy the
    # TRN2 per-instruction wait-slot limits walrus enforces.
    nc = bacc.Bacc()

    xr = nc.dram_tensor("xr", [128, NDC, L], F16, kind="ExternalInput")
    wg = nc.dram_tensor("wg", [128, NDC, EH], F16, kind="ExternalInput")
    wc = nc.dram_tensor("wc", [128, NDC, EH], F16, kind="ExternalInput")
    # bias packed [128, 4]: cols 0..1 = -bg per e-tile, 2..3 = bc per e-tile
    bias = nc.dram_tensor("bias", [128, 2 * NET], FP32, kind="ExternalInput")
    h = nc.dram_tensor("h", [NET, 128, L], F16, kind="ExternalOutput")
    h_pel = h.rearrange("e p l -> p e l")

    op = mybir.AluOpType
    act = mybir.ActivationFunctionType

    with TileContext(nc) as tc:
        with (
            tc.tile_pool(name="consts", bufs=1) as consts,
            tc.tile_pool(name="xpool", bufs=6) as xpool,
            tc.tile_pool(name="work", bufs=3) as work,
            tc.tile_pool(name="hpool", bufs=3) as hpool,
            tc.tile_pool(name="psum", bufs=2, space="PSUM") as psum,
        ):
            # PE warm-up: zero a dummy tile, then issue back-to-back matmuls
            # on it while the first weight/x DMAs are still in flight. PE_HAM
            # sees ~3us of sustained activity and releases the 4/8 clock gate
            # before the real matmul stream begins.
            dummy = consts.tile([128, 128], F16)
            nc.gpsimd.memset(dummy, 0.0)
            warm_ps = psum.tile([128, 2 * NSUB], FP32, tag="pg", name="warm")
            for i in range(N_WARMUP_MM):
                nc.tensor.matmul(
                    warm_ps[:, 0:128], dummy, dummy, start=True, stop=True
                )

            # Sync HWDGE queue order: wg -> x seg 0 -> wc -> x seg 1 -> ...
            # The first matmul group only needs wg + the first x segment.
            # Bias rides the SWDGE (gpsimd) queue.
            wg_sb = consts.tile([128, NDC, EH], F16)
            wc_sb = consts.tile([128, NDC, EH], F16)
            nc.sync.dma_start(wg_sb, wg[:])
            x0_sb = xpool.tile([128, NDC, MAXSEG], F16, tag="x", name="x_0")[
                :, :, : SEGS[0]
            ]
            nc.sync.dma_start(x0_sb, xr[:, :, 0 : SEGS[0]])
            nc.sync.dma_start(wc_sb, wc[:])

            bias_sb = consts.tile([128, 2 * NET], FP32)
            nc.gpsimd.dma_start(bias_sb, bias[:])

            carry = [None] * NET  # [128, 1] AP of the previous h column
            pending_store = None  # (l0, lt, h2 tile) delayed one segment

            l0 = 0
            for t, lt in enumerate(SEGS):
                if t == 0:
                    x_sb = x0_sb
                else:
                    x_sb = xpool.tile(
                        [128, NDC, MAXSEG], F16, tag="x", name=f"x_{t}"
                    )[:, :, :lt]
                    nc.sync.dma_start(x_sb, xr[:, :, l0 : l0 + lt])

                h2 = hpool.tile([128, NET, MAXSEG], F16, tag="h", name=f"h_{t}")
                for et in range(NET):
                    esl = slice(et * 128, (et + 1) * 128)
                    # Flat 2-bank PSUM tiles; each matmul writes one 512-token
                    # bank, the activation reads the whole segment at once.
                    pg = psum.tile([128, 2 * NSUB], FP32, tag="pg", name=f"pg{et}_{t}")
                    pc = psum.tile([128, 2 * NSUB], FP32, tag="pc", name=f"pc{et}_{t}")
                    for n0 in range(0, lt, NSUB):
                        w = min(NSUB, lt - n0)
                        nsl = slice(n0, n0 + w)
                        for dc in range(NDC):
                            nc.tensor.matmul(
                                pg[:, n0 : n0 + w],
                                wg_sb[:, dc, esl],
                                x_sb[:, dc, nsl],
                                start=(dc == 0),
                                stop=(dc == NDC - 1),
                            )
                        for dc in range(NDC):
                            nc.tensor.matmul(
                                pc[:, n0 : n0 + w],
                                wc_sb[:, dc, esl],
                                x_sb[:, dc, nsl],
                                start=(dc == 0),
                                stop=(dc == NDC - 1),
                            )
                    # a = sigmoid(-(z_g + bg)) = 1 - g ; c = tanh(z_c + bc)
                    a_t = work.tile([128, MAXSEG], F16, tag=f"a{et}", name=f"a{et}_{t}")[
                        :, :lt
                    ]
                    c_t = work.tile([128, MAXSEG], F16, tag=f"c{et}", name=f"c{et}_{t}")[
                        :, :lt
                    ]
                    nc.scalar.activation(
                        a_t, pg[:, :lt], act.Sigmoid,
                        bias=bias_sb[:, et : et + 1], scale=-1.0,
                    )
                    nc.scalar.activation(
                        c_t, pc[:, :lt], act.Tanh,
                        bias=bias_sb[:, NET + et : NET + et + 1], scale=1.0,
                    )
                    # bneg = (a - 1) * c = -g * c
                    bn_t = work.tile(
                        [128, MAXSEG], F16, tag=f"b{et}", name=f"b{et}_{t}"
                    )[:, :lt]
                    if BN_ON_GPSIMD[et]:
                        # Pool rejects TensorScalarPtr; use two TensorTensor
                        # ops: t = a*c, bn = t - c.
                        t_t = work.tile(
                            [128, MAXSEG], F16, tag=f"t{et}", name=f"t{et}_{t}"
                        )[:, :lt]
                        nc.gpsimd.tensor_mul(t_t, a_t, c_t)
                        nc.gpsimd.tensor_sub(bn_t, t_t, c_t)
                    else:
                        nc.vector.scalar_tensor_tensor(
                            bn_t, a_t, 1.0, c_t, op.subtract, op.mult
                        )
                    # h = a * h_prev - bneg  (fp32 state in HW, fp16 storage)
                    init = 0.0 if carry[et] is None else carry[et]
                    nc.vector.tensor_tensor_scan(
                        h2[:, et, :lt], a_t, bn_t, init, op.mult, op.subtract
                    )
                    carry[et] = h2[:, et, lt - 1 : lt]
                # One store per segment, both e-tiles, emitted one segment
                # late so gpsimd never head-of-line blocks on this scan.
                if pending_store is not None:
                    pl0, plt, ph2 = pending_store
                    nc.gpsimd.dma_start(
                        h_pel[:, :, pl0 : pl0 + plt], ph2[:, :, :plt]
                    )
                pending_store = (l0, lt, h2)
                l0 += lt
            pl0, plt, ph2 = pending_store
            nc.gpsimd.dma_start(h_pel[:, :, pl0 : pl0 + plt], ph2[:, :, :plt])
    return nc


def _in_maps(x, Wg, bg, Wc, bc):
    maps = []
    xr = {}
    for c in range(NCORES):
        b, eh = c // 2, c % 2
        e0 = eh * EH
        if b not in xr:
            # [L, D] -> [D, L] -> [dc, p, L] -> [p, dc, L] fp16
            xr[b] = x[b].T.reshape(NDC, 128, L).transpose(1, 0, 2).astype(np.float16)
        bias_pack = np.concatenate(
            [
                (-bg[e0 : e0 + EH]).reshape(NET, 128).T,
                bc[e0 : e0 + EH].reshape(NET, 128).T,
            ],
            axis=1,
        ).astype(np.float32)
        maps.append(
            {
                "xr": xr[b],
                "wg": Wg[e0 : e0 + EH].T.reshape(NDC, 128, EH)
                .transpose(1, 0, 2).astype(np.float16),
                "wc": Wc[e0 : e0 + EH].T.reshape(NDC, 128, EH)
                .transpose(1, 0, 2).astype(np.float16),
                "bias": np.ascontiguousarray(bias_pack),
            }
        )
    return maps


def kernel(x, Wg, bg, Wc, bc):
    global _last_results
    x = np.asarray(x, dtype=np.float32)
    Wg = np.asarray(Wg, dtype=np.float32)
    bg = np.asarray(bg, dtype=np.float32)
    Wc = np.asarray(Wc, dtype=np.float32)
    bc = np.asarray(bc, dtype=np.float32)

    nc = build_nc()
    if not nc.is_finalized():
        nc.finalize()
    res = run_bass_kernel_spmd(
        nc,
        _in_maps(x, Wg, bg, Wc, bc),
        list(range(NCORES)),
        tmpdir=os.environ.get("KERNEL_TMPDIR"),
    )
    _last_results = res

    out = np.empty((B, L, D), dtype=np.float32)
    for b in range(B):
        hb = np.concatenate(
            [
                res.results[2 * b]["h"].reshape(EH, L),
                res.results[2 * b + 1]["h"].reshape(EH, L),
            ],
            axis=0,
        ).astype(np.float32)
        out[b] = hb.T
    return out


# revision 6
# speedup vs baseline: 1.2362x; 1.2362x over previous
"""MinGRU kernel for Trainium2 (8 NeuronCores, Bass/Tile) — v3.

Reference computation (B=4, L=8192, D=512, fp32):
    gates = sigmoid(x @ Wg.T + bg)
    cands = tanh(x @ Wc.T + bc)
    h_t   = (1 - g_t) * h_{t-1} + g_t * c_t   (scan along L, h_0 = 0)

Sharding: core c -> (batch b = c//2, channel half = c%2). Each core computes
its batch's full L range for 256 of the 512 output channels; the scan along L
is per (b, channel) so no cross-core communication is needed.

Measured engine budget per core (v1 trace): PE 54.6us (fixed roofline:
131072 PE rows), DVE scan 2.09ns/elem + bneg STT 1.04ns/elem over 16384
elems/lane = 51us payload — DVE is the co-bottleneck, so the design
minimizes DVE instruction count and keeps the dependency graph
single-chain (v2's DVE<->GpSimd ping-pong doubled semaphore costs and
regressed; GpSimd tensor ops run at ~2ns/elem + ~570ns/op and are not
worth it).

v3 vs v1 (90.5us):
  * x and W cast to fp16 on the host: input DMA halves (16.8 -> 8.9 MB/core);
    the x feed (33us queue-wall) ducks well under the PE roofline.
  * Scan units of 2048 tokens: one STT + one scan per (unit, e-tile) with
    matmul/ACT filling the unit in 1024-token halves ([128,1024] fp32 PSUM
    tiles = 2 banks, tags g/c x bufs 2 = 8 banks, still double-buffered).
    Fewer DVE ops -> less fixed overhead and fewer semaphores.
  * Activations read the full 1024-token PSUM tile in one instruction.
  * 12 warm-up matmuls on a zeroed dummy tile while the first weight/x DMAs
    fly: PE_HAM releases the 4/8 cold clock gate before real matmuls start.
  * First segment 512 tokens so the x prefetch stays ahead of the PE from
    the start (v2's 256-token head caused 6us of early PE gaps).
  * -bg negated on the host; h stored fp16 [2, 128, L] and upcast on host.
"""

import os
import sys

sys.path.insert(0, "/opt/trn_rl_repo")

import numpy as np

import concourse.bacc as bacc
import concourse.bass as bass
import concourse.mybir as mybir
from concourse.bass_utils import run_bass_kernel_spmd
from concourse.tile import TileContext

B, L, D = 4, 8192, 512
NCORES = 8
EH = D // 2          # output channels per core
NET = EH // 128      # e-tiles per core (2)
NDC = D // 128       # contraction chunks (4)
NSUB = 512           # one fp32 PSUM bank of tokens (matmul N limit)
PSEG = 1024          # tokens per PSUM tile / ACT instruction
# Scan units: one STT + scan per unit; matmul/ACT work in <=1024 chunks.
SEGS = [512, 512, 2048, 2048, 2048, 512, 256, 256]
assert sum(SEGS) == L
MAXSEG = max(SEGS)

FP32 = mybir.dt.float32
F16 = mybir.dt.float16
_last_results = None

N_WARMUP_MM = 12


def build_nc() -> bass.Bass:
    # Bacc (not plain Bass): its compile() runs move_matmul_waits_to_ldweights
    # and generate_event_semaphores, which split multi-sem waits to satisfy the
    # TRN2 per-instruction wait-slot limits walrus enforces.
    nc = bacc.Bacc()

    xr = nc.dram_tensor("xr", [128, NDC, L], F16, kind="ExternalInput")
    wg = nc.dram_tensor("wg", [128, NDC, EH], F16, kind="ExternalInput")
    wc = nc.dram_tensor("wc", [128, NDC, EH], F16, kind="ExternalInput")
    # bias packed [128, 4]: cols 0..1 = -bg per e-tile, 2..3 = bc per e-tile
    bias = nc.dram_tensor("bias", [128, 2 * NET], FP32, kind="ExternalInput")
    h = nc.dram_tensor("h", [NET, 128, L], F16, kind="ExternalOutput")
    h_pel = h.rearrange("e p l -> p e l")

    op = mybir.AluOpType
    act = mybir.ActivationFunctionType

    with TileContext(nc) as tc:
        with (
            tc.tile_pool(name="consts", bufs=1) as consts,
            tc.tile_pool(name="xpool", bufs=4) as xpool,
            tc.tile_pool(name="work", bufs=2) as work,
            tc.tile_pool(name="hpool", bufs=2) as hpool,
            tc.tile_pool(name="psum", bufs=2, space="PSUM") as psum,
        ):
            # PE warm-up: zero a dummy tile, then issue back-to-back matmuls
            # on it while the first weight/x DMAs are still in flight, so
            # PE_HAM releases the 4/8 cold clock gate before the real stream.
            dummy = consts.tile([128, 128], F16)
            nc.gpsimd.memset(dummy, 0.0)
            warm_ps = psum.tile([128, PSEG], FP32, tag="pg", name="warm")
            for _ in range(N_WARMUP_MM):
                nc.tensor.matmul(
                    warm_ps[:, 0:128], dummy, dummy, start=True, stop=True
                )

            # Sync HWDGE queue order: wg -> x seg 0 -> wc -> x seg 1 -> ...
            # The first matmul group only needs wg + the first x segment.
            # Bias rides the SWDGE (gpsimd) queue.
            wg_sb = consts.tile([128, NDC, EH], F16)
            wc_sb = consts.tile([128, NDC, EH], F16)
            nc.sync.dma_start(wg_sb, wg[:])
            x0_sb = xpool.tile([128, NDC, MAXSEG], F16, tag="x", name="x_0")[
                :, :, : SEGS[0]
            ]
            nc.sync.dma_start(x0_sb, xr[:, :, 0 : SEGS[0]])
            nc.sync.dma_start(wc_sb, wc[:])

            bias_sb = consts.tile([128, 2 * NET], FP32)
            nc.gpsimd.dma_start(bias_sb, bias[:])

            carry = [None] * NET  # [128, 1] AP of the previous h column

            l0 = 0
            for t, lt in enumerate(SEGS):
                if t == 0:
                    x_sb = x0_sb
                else:
                    x_sb = xpool.tile(
                        [128, NDC, MAXSEG], F16, tag="x", name=f"x_{t}"
                    )[:, :, :lt]
                    nc.sync.dma_start(x_sb, xr[:, :, l0 : l0 + lt])

                h2 = hpool.tile([128, NET, MAXSEG], F16, tag="h", name=f"h_{t}")
                for et in range(NET):
                    esl = slice(et * 128, (et + 1) * 128)
                    a_t = work.tile(
                        [128, MAXSEG], F16, tag=f"a{et}", name=f"a{et}_{t}"
                    )[:, :lt]
                    c_t = work.tile(
                        [128, MAXSEG], F16, tag=f"c{et}", name=f"c{et}_{t}"
                    )[:, :lt]
                    # 1024-token PSUM passes fill the scan unit.
                    for p0 in range(0, lt, PSEG):
                        pw = min(PSEG, lt - p0)
                        pg = psum.tile(
                            [128, PSEG], FP32, tag="pg", name=f"pg{et}_{t}_{p0}"
                        )
                        pc = psum.tile(
                            [128, PSEG], FP32, tag="pc", name=f"pc{et}_{t}_{p0}"
                        )
                        for n0 in range(0, pw, NSUB):
                            w = min(NSUB, pw - n0)
                            xsl = slice(p0 + n0, p0 + n0 + w)
                            for dc in range(NDC):
                                nc.tensor.matmul(
                                    pg[:, n0 : n0 + w],
                                    wg_sb[:, dc, esl],
                                    x_sb[:, dc, xsl],
                                    start=(dc == 0),
                                    stop=(dc == NDC - 1),
                                )
                            for dc in range(NDC):
                                nc.tensor.matmul(
                                    pc[:, n0 : n0 + w],
                                    wc_sb[:, dc, esl],
                                    x_sb[:, dc, xsl],
                                    start=(dc == 0),
                                    stop=(dc == NDC - 1),
                                )
                        # a = sigmoid(-(z_g + bg)) = 1 - g ; c = tanh(z_c + bc)
                        nc.scalar.activation(
                            a_t[:, p0 : p0 + pw], pg[:, :pw], act.Sigmoid,
                            bias=bias_sb[:, et : et + 1], scale=-1.0,
                        )
                        nc.scalar.activation(
                            c_t[:, p0 : p0 + pw], pc[:, :pw], act.Tanh,
                            bias=bias_sb[:, NET + et : NET + et + 1], scale=1.0,
                        )
                    # bneg = (a - 1) * c = -g * c  (one DVE op per unit)
                    bn_t = work.tile(
                        [128, MAXSEG], F16, tag=f"b{et}", name=f"b{et}_{t}"
                    )[:, :lt]
                    nc.vector.scalar_tensor_tensor(
                        bn_t, a_t, 1.0, c_t, op.subtract, op.mult
                    )
                    # h = a * h_prev - bneg  (fp32 state in HW, fp16 storage)
                    init = 0.0 if carry[et] is None else carry[et]
                    nc.vector.tensor_tensor_scan(
                        h2[:, et, :lt], a_t, bn_t, init, op.mult, op.subtract
                    )
                    carry[et] = h2[:, et, lt - 1 : lt]
                # One store per unit covering both e-tiles (SWDGE; gpsimd has
                # nothing else queued, so blocking on the scans is free).
                nc.gpsimd.dma_start(h_pel[:, :, l0 : l0 + lt], h2[:, :, :lt])
                l0 += lt
    return nc


def _in_maps(x, Wg, bg, Wc, bc):
    maps = []
    xr = {}
    for c in range(NCORES):
        b, eh = c // 2, c % 2
        e0 = eh * EH
        if b not in xr:
            # [L, D] -> [D, L] -> [dc, p, L] -> [p, dc, L] fp16
            xr[b] = x[b].T.reshape(NDC, 128, L).transpose(1, 0, 2).astype(np.float16)
        bias_pack = np.concatenate(
            [
                (-bg[e0 : e0 + EH]).reshape(NET, 128).T,
                bc[e0 : e0 + EH].reshape(NET, 128).T,
            ],
            axis=1,
        ).astype(np.float32)
        maps.append(
            {
                "xr": xr[b],
                "wg": Wg[e0 : e0 + EH].T.reshape(NDC, 128, EH)
                .transpose(1, 0, 2).astype(np.float16),
                "wc": Wc[e0 : e0 + EH].T.reshape(NDC, 128, EH)
                .transpose(1, 0, 2).astype(np.float16),
                "bias": np.ascontiguousarray(bias_pack),
            }
        )
    return maps


def kernel(x, Wg, bg, Wc, bc):
    global _last_results
    x = np.asarray(x, dtype=np.float32)
    Wg = np.asarray(Wg, dtype=np.float32)
    bg = np.asarray(bg, dtype=np.float32)
    Wc = np.asarray(Wc, dtype=np.float32)
    bc = np.asarray(bc, dtype=np.float32)

    nc = build_nc()
    if not nc.is_finalized():
        nc.finalize()
    res = run_bass_kernel_spmd(
        nc,
        _in_maps(x, Wg, bg, Wc, bc),
        list(range(NCORES)),
        tmpdir=os.environ.get("KERNEL_TMPDIR"),
    )
    _last_results = res

    out = np.empty((B, L, D), dtype=np.float32)
    for b in range(B):
        hb = np.concatenate(
            [
                res.results[2 * b]["h"].reshape(EH, L),
                res.results[2 * b + 1]["h"].reshape(EH, L),
            ],
            axis=0,
        ).astype(np.float32)
        out[b] = hb.T
    return out
